# revision 39
# baseline (speedup 1.0000x reference)
"""Trainium2 Bass kernel for CapsNet conv + dynamic-routing block.

Math note: in the reference, `pred` has a singleton MI axis, so the
softmax-weighted sum over MI is `pred` itself for any routing logits
(softmax rows sum to 1), and the `b` updates never change `c`.  The whole
module therefore reduces exactly to

    out = squash(conv2d_3x3(x2, conv_w) + conv_b)   # squash over DO

with x2 = x reshaped [B, MI*DI, H, W] and output [B, MO, H, W, DO].

Strategy: data-parallel over batch (1 image per NeuronCore, 8 cores).
Conv runs in bf16 (tolerance 2e-2; bf16 conv lands ~5e-3): halves DMA
traffic, transposes at 1.0 cyc/row.  Per core:
  1. A dozen tiny warm-up matmuls on a zeroed tile start the PE p-state
     ramp while the first DMAs are still in flight.
  2. x[b] arrives as 8 row-group DMAs into a zero-padded [128, 66, 66]
     bf16 image; weights as 3 kh-slice DMAs so tap-0 weights land first.
  3. 3x3 conv, one 512-pixel chunk at a time: 9 accumulating bf16
     matmuls (lhsT = W[s][ci,co] stationary, rhs = shifted window of
     x_pad), f32 PSUM.  Chunk c's post-processing is emitted after chunk
     c+1's matmuls so the PE transposes never stall the conv stream.
  4. post: ACT bias-add (casts bf16) -> PE transpose to [pix, co] ->
     Pool square -> DVE sum over DO -> ACT sqrt -> DVE factor chain ->
     DVE scale -> bf16 store per chunk (host casts back to f32).  The
     factor chain for chunk c is emitted after chunk c+1's square so the
     ACT sqrt round-trip never idles the DVE.
"""

from contextlib import ExitStack

import numpy as np
import ml_dtypes

import concourse.bass as bass
import concourse.mybir as mybir
import concourse.tile as tile
from concourse import bacc
from concourse.bass_utils import run_bass_kernel_spmd
from concourse.masks import make_identity

B, MI, H, W, DI = 8, 8, 64, 64, 16
MO, DO = 8, 16
CI = MI * DI  # 128
CO = MO * DO  # 128
P = 128
NCHUNK = 8  # 512-pixel chunks per 64x64 image
XFLAT = 66 + 65 * H + 66  # flat x layout: lead pad + 65-wide rows + tail pad
EPS = 1e-7

F32 = mybir.dt.float32
BF16 = mybir.dt.bfloat16


def _body(tc, x_in, w_in, b_in, out_d, reps=1):
    nc = tc.nc
    with ExitStack() as ctx:
        consts = ctx.enter_context(tc.tile_pool(name="consts", bufs=1))
        cpsum = ctx.enter_context(tc.tile_pool(name="cpsum", bufs=3, space="PSUM"))
        opsum = ctx.enter_context(tc.tile_pool(name="opsum", bufs=4, space="PSUM"))
        wpsum = ctx.enter_context(tc.tile_pool(name="wpsum", bufs=1, space="PSUM"))
        work = ctx.enter_context(tc.tile_pool(name="work", bufs=3))

        # p-state warm-up: tiny matmuls on a zeroed tile anchor the PE's
        # DVFS ramp clock while the x/w DMAs are still in flight.
        wrm = consts.tile([P, 64], BF16)
        nc.vector.memset(wrm[:], 0.0)
        wps = wpsum.tile([8, 64], F32)
        for _ in range(12):
            nc.tensor.matmul(wps[:], wrm[:, :8], wrm[:], start=True, stop=True)

        # weights: [ci, s, co] bf16, split so taps 0-2 arrive first (ACT
        # HWDGE ring; HWDGE dispatch is ~630ns per DMA, so keep DMA count low)
        w_sb = consts.tile([P, 9, CO], BF16)
        nc.scalar.dma_start(w_sb[:, :3, :], w_in[:, :3, :])
        nc.scalar.dma_start(w_sb[:, 3:, :], w_in[:, 3:, :])

        bias_sb = consts.tile([P, 1], F32)
        nc.scalar.dma_start(bias_sb[:], b_in)

        # x image in a flat 65-wide-row layout: the host appends one zero
        # column per row (w'=64), which serves as BOTH the left pad of the
        # next row and the right pad of the current one; 66 leading and
        # trailing zeros cover the vertical pads.  Rows are contiguous in
        # DRAM and SBUF, so every load runs at full DMA descriptor rate.
        #   flat(h', w') = 66 + 65*h' + w'
        xflat = consts.tile([P, XFLAT], BF16)
        nc.vector.memset(xflat[:, :66], 0.0)
        nc.vector.memset(xflat[:, 66 + 65 * H :], 0.0)

        identity_f32 = consts.tile([P, P], F32)
        make_identity(nc, identity_f32[:])
        identity_bf = consts.tile([P, P], BF16)
        nc.scalar.copy(identity_bf[:], identity_f32[:])

        eps_sb = consts.tile([P, 1], F32)
        nc.vector.memset(eps_sb[:], EPS)

        # x loads in 3 row-groups (contiguous in BOTH source and dest);
        # group g covers rows [XROW[g], XROW[g+1])
        XROW = [0, 18, 41, 64]

        def load_group(g):
            # group 0 on the SP ring (runs first); later groups on the ACT
            # ring BEHIND the weight slices, so the shared DMA pipe serves
            # e0, w(0:3), w(3:9), g1, g2 in exactly that order.
            a, b = 65 * XROW[g], 65 * XROW[g + 1]
            ring = nc.sync if g == 0 else nc.scalar
            ring.dma_start(xflat[:, 66 + a : 66 + b], x_in[:, a:b])

        out_sb = consts.tile([P, NCHUNK, 4, CO], BF16)

        def conv_chunk(c):
            ps = cpsum.tile([P, 4 * P], F32, tag="ps")
            for s in range(9):
                kh, kw = s // 3, s % 3
                off = 65 * (8 * c + kh) + kw
                rhs = xflat[:, off : off + 520].rearrange(
                    "p (r w) -> p r w", w=65
                )[:, :, :64]
                nc.tensor.matmul(
                    ps[:], w_sb[:, s, :], rhs, start=(s == 0), stop=(s == 8)
                )
            return ps

        def bias_chunk(ps, halves=False):
            # PSUM -> SBUF with bias add (ACT, per-partition bias AP),
            # casting to bf16 so the transposes run at 1.0 cyc/row.
            # halves=True emits two half-width adds so the tail transposes
            # can start after only half the copy.
            s_sb = work.tile([P, 4 * P], BF16, tag="s_sb")
            if halves:
                nc.scalar.add(s_sb[:, : 2 * P], ps[:, : 2 * P], bias_sb[:])
                nc.scalar.add(s_sb[:, 2 * P :], ps[:, 2 * P :], bias_sb[:])
            else:
                nc.scalar.add(s_sb[:], ps[:], bias_sb[:])
            return s_sb

        def t_part(c, s_sb, t0=0, t1=4):
            # transpose [t0:t1] 128-px blocks to [pix, co] (PE)
            nt = t1 - t0
            so_full = opsum.tile([P, 4, P], BF16, tag="so")
            so = so_full[:, :nt, :]
            for t in range(nt):
                nc.tensor.transpose(
                    so[:, t, :], s_sb[:, (t0 + t) * P : (t0 + t + 1) * P],
                    identity_bf[:],
                )
            return so

        def sq_part(c, so, t0=0, t1=4):
            # square (ACT); sum over DO (DVE)
            nt = t1 - t0
            sq = work.tile([P, nt, P], BF16, tag=f"sq{nt}")
            nc.scalar.square(sq[:], so[:])
            red = work.tile([P, nt * MO], BF16, tag=f"red{nt}")
            with nc.allow_low_precision(reason="squash norm tolerates bf16"):
                nc.vector.tensor_reduce(
                    red[:],
                    sq[:].rearrange("p t (g do) -> p (t g) do", do=DO),
                    axis=mybir.AxisListType.X,
                    op=mybir.AluOpType.add,
                )
            return red

        def post_a(c, s_sb, t0=0, t1=4):
            so = t_part(c, s_sb, t0, t1)
            red = sq_part(c, so, t0, t1)
            return so, red

        def post_b(c, so, red, t0=0, t1=4, tail=False, mul_engine=None, ring=None):
            # factor = red / ((1+red) * sqrt(red+eps)); scale + store.
            # The ACT sqrt sits two pipeline stages behind its reduce, so
            # neither ACT nor DVE ever stalls on the cross-engine hop.
            nt = t1 - t0
            r = work.tile([P, nt * MO], BF16, tag=f"r{nt}")
            nc.scalar.activation(
                r[:], red[:], mybir.ActivationFunctionType.Sqrt, bias=eps_sb[:]
            )
            d = work.tile([P, nt * MO], BF16, tag=f"d{nt}")
            nc.vector.scalar_tensor_tensor(
                d[:], red[:], 1.0, r[:], mybir.AluOpType.add, mybir.AluOpType.mult
            )
            rcp = work.tile([P, nt * MO], BF16, tag=f"rcp{nt}")
            with nc.allow_low_precision(reason="squash factor tolerates bf16"):
                nc.vector.reciprocal(rcp[:], d[:])
            fac = work.tile([P, nt * MO], BF16, tag=f"fac{nt}")
            nc.vector.tensor_mul(fac[:], red[:], rcp[:])
            # (Pool cannot read PSUM on TRN2, so the scale stays on DVE)
            (mul_engine or nc.vector).tensor_mul(
                out_sb[:, c, t0:t1].rearrange("p t (g do) -> p (t g) do", do=DO),
                so.rearrange("p t (g do) -> p (t g) do", do=DO),
                fac[:, :, None].to_broadcast((P, nt * MO, DO)),
            )
            # mid-stream stores ride the software DGE (Pool engine is idle
            # there and a SEQ-blocking wait is harmless); tail stores use
            # HWDGE rings (ACT/SP), which have no conflicting work left.
            if tail:
                (ring or nc.scalar).dma_start(out_d[:, c, t0:t1], out_sb[:, c, t0:t1])
            else:
                nc.gpsimd.dma_start(out_d[:, c, t0:t1], out_sb[:, c, t0:t1])

        def one_image():
            # Emission per chunk: mm(c), post_a(c-1), bias(c), post_b(c-2)
            # — a depth-2 software pipeline; the last chunk posts in halves.
            load_group(0)
            load_group(1)
            sbufs = {}  # c -> s_sb
            a_state = {}  # c -> (so, red)
            for c in range(NCHUNK - 1):
                if c == 1:
                    load_group(2)
                ps = conv_chunk(c)
                # post_a(c-1) BEFORE bias(c) on the ACT queue: T(c-1) runs
                # at mm(c)'s end, so square(c-1) is ready right away and
                # the DVE reduce isn't pushed a chunk late.  post_b(c-1)
                # directly after (depth-1 pipeline): the DVE absorbs the
                # sqrt round-trip mid-stream, so no post backlog remains
                # when the matmul stream drains.
                if c >= 1:
                    a_state[c - 1] = post_a(c - 1, sbufs.pop(c - 1))
                sbufs[c] = bias_chunk(ps)
                if c >= 1:
                    post_b(c - 1, *a_state.pop(c - 1))
            # drain: chunk 7 biases and posts in half-chunks so the tail's
            # serial chain is half-width all the way down.  The bias halves
            # go FIRST on the ACT queue (they gate the tail transposes);
            # squares/sqrts slot in behind them.
            ps7 = conv_chunk(7)
            so6 = t_part(6, sbufs.pop(6))
            s7 = bias_chunk(ps7, halves=True)
            red6 = sq_part(6, so6)
            so7a = t_part(7, s7, 0, 2)
            post_b(6, so6, red6, tail=True, ring=nc.scalar,
                   mul_engine=nc.vector)
            red7a = sq_part(7, so7a, 0, 2)
            so7b = t_part(7, s7, 2, 4)
            post_b(7, so7a, red7a, 0, 2, tail=True, ring=nc.sync)
            red7b = sq_part(7, so7b, 2, 4)
            post_b(7, so7b, red7b, 2, 4, tail=True, ring=nc.scalar,
                   mul_engine=nc.vector)

        if reps == 1:
            one_image()
        else:
            with tc.For_i(0, reps, 1):
                one_image()


_NC_CACHE = {}


def _get_nc(reps=1):
    key = ("nc", reps)
    if key not in _NC_CACHE:
        nc = bacc.Bacc("TRN2", target_bir_lowering=False, debug=False, num_devices=8)
        x_in = nc.dram_tensor("x", [CI, H * 65], BF16, kind="ExternalInput").ap()
        w_in = nc.dram_tensor("w", [CI, 9, CO], BF16, kind="ExternalInput").ap()
        b_in = nc.dram_tensor("bias", [CO, 1], F32, kind="ExternalInput").ap()
        out_d = nc.dram_tensor("out", [P, NCHUNK, 4, CO], BF16, kind="ExternalOutput").ap()
        with tile.TileContext(nc) as tc:
            _body(tc, x_in, w_in, b_in, out_d, reps=reps)
        nc.compile()
        _NC_CACHE[key] = nc
    return _NC_CACHE[key]


def run(x, conv_w, conv_b, trace=False, reps=1):
    nc = _get_nc(reps=reps)
    # shard/prep: channel-major x per image with one zero column appended
    # per row (the device's 65-wide-row virtual-padding layout), bf16
    x4 = (
        np.asarray(x, dtype=np.float32)
        .transpose(0, 1, 4, 2, 3)
        .reshape(B, CI, H, W)
        .astype(ml_dtypes.bfloat16)
    )
    xt = np.zeros((B, CI, H, 65), dtype=ml_dtypes.bfloat16)
    xt[..., :W] = x4
    xt = np.ascontiguousarray(xt.reshape(B, CI, H * 65))
    w9 = np.ascontiguousarray(
        np.asarray(conv_w, dtype=np.float32)
        .reshape(CO, CI, 9)
        .transpose(1, 2, 0)
        .astype(ml_dtypes.bfloat16)
    )
    bias = np.ascontiguousarray(np.asarray(conv_b, dtype=np.float32).reshape(CO, 1))
    in_maps = [{"x": xt[b], "w": w9, "bias": bias} for b in range(B)]
    res = run_bass_kernel_spmd(nc, in_maps, list(range(B)), trace=trace)
    # gather/unshard: out_dev[p, c, t, mo, do] -> out[b, mo, h, w, do]
    # with h = 8c + 2t + p//64, w = p%64
    dev = np.stack(
        [res.results[i]["out"].astype(np.float32) for i in range(B)], axis=0
    )
    dev = dev.reshape(B, 2, W, NCHUNK, 4, MO, DO)  # [b, hl, w, c, t, mo, do]
    out = np.ascontiguousarray(
        dev.transpose(0, 5, 3, 4, 1, 2, 6).reshape(B, MO, H, W, DO)
    )
    return out, res


def kernel(x, conv_w, conv_b, b_logits=None, **_ignored):
    # b_logits provably has no effect on the reference output (see module
    # docstring), so it is accepted and ignored.
    out, _ = run(x, conv_w, conv_b, trace=False)
    return out


# revision 42
# speedup vs baseline: 1.2145x; 1.2145x over previous
"""Trainium2 Bass kernel for CapsNet conv + dynamic-routing block.

Math note: in the reference, `pred` has a singleton MI axis, so the
softmax-weighted sum over MI is `pred` itself for any routing logits
(softmax rows sum to 1), and the `b` updates never change `c`.  The whole
module therefore reduces exactly to

    out = squash(conv2d_3x3(x2, conv_w) + conv_b)   # squash over DO

with x2 = x reshaped [B, MI*DI, H, W] and output [B, MO, H, W, DO].

Strategy: data-parallel over batch (1 image per NeuronCore, 8 cores).
Conv runs in bf16 (tolerance 2e-2; bf16 conv lands ~5e-3): halves DMA
traffic, transposes at 1.0 cyc/row.  Per core:
  1. A dozen tiny warm-up matmuls on a zeroed tile anchor the PE p-state
     ramp clock each iteration while the DMAs are still in flight.
  2. x[b] arrives in a flat 65-wide-row layout (host appends one zero
     column per row, serving as both horizontal pads; 66 lead/trail
     zeros give the vertical pads) — fully contiguous DMA at max
     descriptor rate, 3 row-group loads on the SP ring; weights as two
     slices on the ACT ring so tap-0 weights land first.
  3. 3x3 conv, one 512-pixel chunk at a time: 9 accumulating bf16
     matmuls (lhsT = W[s][ci,co] stationary, rhs = strided window of the
     flat image), f32 PSUM.  Chunk c's post-processing is emitted after
     chunk c+1's matmuls so the PE transposes never stall the conv
     stream.
  4. post: ACT bias-add (casts bf16) -> PE transpose to [pix, co] ->
     ACT square -> DVE sum over DO (bf16) -> ACT sqrt -> DVE factor
     chain -> DVE scale -> bf16 store per chunk (host casts back to
     f32).  Mid-stream stores ride the Pool SWDGE; the tail (chunk 7 in
     half-chunks) stores on the ACT ring, so no store trigger can delay
     the next iteration's loads.
"""

from contextlib import ExitStack

import numpy as np
import ml_dtypes

import concourse.bass as bass
import concourse.mybir as mybir
import concourse.tile as tile
from concourse import bacc
from concourse.bass_utils import run_bass_kernel_spmd
from concourse.masks import make_identity

B, MI, H, W, DI = 8, 8, 64, 64, 16
MO, DO = 8, 16
CI = MI * DI  # 128
CO = MO * DO  # 128
P = 128
NCHUNK = 8  # 512-pixel chunks per 64x64 image
XFLAT = 66 + 65 * H + 66  # flat x layout: lead pad + 65-wide rows + tail pad
EPS = 1e-7

F32 = mybir.dt.float32
BF16 = mybir.dt.bfloat16


def _body(tc, x_in, w_in, b_in, out_d, reps=1):
    nc = tc.nc
    with ExitStack() as ctx:
        consts = ctx.enter_context(tc.tile_pool(name="consts", bufs=1))
        cpsum = ctx.enter_context(tc.tile_pool(name="cpsum", bufs=3, space="PSUM"))
        opsum = ctx.enter_context(tc.tile_pool(name="opsum", bufs=4, space="PSUM"))
        wpsum = ctx.enter_context(tc.tile_pool(name="wpsum", bufs=1, space="PSUM"))
        work = ctx.enter_context(tc.tile_pool(name="work", bufs=3))

        # p-state warm-up tile: tiny matmuls on this zeroed tile anchor
        # the PE's DVFS ramp clock while each iteration's DMAs are in
        # flight (emitted at the top of one_image so looped runs re-warm).
        wrm = consts.tile([P, 64], BF16)
        nc.vector.memset(wrm[:], 0.0)

        # weights: [ci, s, co] bf16, split so taps 0-2 arrive first (ACT
        # HWDGE ring; HWDGE dispatch is ~630ns per DMA, so keep DMA count low)
        w_sb = consts.tile([P, 9, CO], BF16)
        nc.scalar.dma_start(w_sb[:, :6, :], w_in[:, :6, :])
        nc.scalar.dma_start(w_sb[:, 6:, :], w_in[:, 6:, :])

        bias_sb = consts.tile([P, 1], F32)
        nc.scalar.dma_start(bias_sb[:], b_in)

        # x image in a flat 65-wide-row layout: the host appends one zero
        # column per row (w'=64), which serves as BOTH the left pad of the
        # next row and the right pad of the current one; 66 leading and
        # trailing zeros cover the vertical pads.  Rows are contiguous in
        # DRAM and SBUF, so every load runs at full DMA descriptor rate.
        #   flat(h', w') = 66 + 65*h' + w'
        xflat = consts.tile([P, XFLAT], BF16)
        nc.vector.memset(xflat[:, :66], 0.0)
        nc.vector.memset(xflat[:, 66 + 65 * H :], 0.0)

        identity_f32 = consts.tile([P, P], F32)
        make_identity(nc, identity_f32[:])
        identity_bf = consts.tile([P, P], BF16)
        nc.scalar.copy(identity_bf[:], identity_f32[:])

        eps_sb = consts.tile([P, 1], F32)
        nc.vector.memset(eps_sb[:], EPS)

        # x loads in 3 row-groups (contiguous in BOTH source and dest);
        # group g covers rows [XROW[g], XROW[g+1])
        XROW = [0, 18, 41, 64]

        def load_group(g):
            # all x loads on the SP ring: the ACT ring carries the weight
            # slices (iteration 0) and the tail stores, so neither ring's
            # store trigger can ever delay the next iteration's loads.
            a, b = 65 * XROW[g], 65 * XROW[g + 1]
            nc.sync.dma_start(xflat[:, 66 + a : 66 + b], x_in[:, a:b])

        out_sb = consts.tile([P, NCHUNK, 4, CO], BF16)

        def conv_chunk(c):
            ps = cpsum.tile([P, 4 * P], F32, tag="ps")
            for s in range(9):
                kh, kw = s // 3, s % 3
                off = 65 * (8 * c + kh) + kw
                rhs = xflat[:, off : off + 520].rearrange(
                    "p (r w) -> p r w", w=65
                )[:, :, :64]
                nc.tensor.matmul(
                    ps[:], w_sb[:, s, :], rhs, start=(s == 0), stop=(s == 8)
                )
            return ps

        def bias_chunk(ps, halves=False):
            # PSUM -> SBUF with bias add (ACT, per-partition bias AP),
            # casting to bf16 so the transposes run at 1.0 cyc/row.
            # halves=True emits two half-width adds so the tail transposes
            # can start after only half the copy.
            s_sb = work.tile([P, 4 * P], BF16, tag="s_sb")
            if halves:
                nc.scalar.add(s_sb[:, : 2 * P], ps[:, : 2 * P], bias_sb[:])
                nc.scalar.add(s_sb[:, 2 * P :], ps[:, 2 * P :], bias_sb[:])
            else:
                nc.scalar.add(s_sb[:], ps[:], bias_sb[:])
            return s_sb

        def t_part(c, s_sb, t0=0, t1=4):
            # transpose [t0:t1] 128-px blocks to [pix, co] (PE)
            nt = t1 - t0
            so_full = opsum.tile([P, 4, P], BF16, tag="so")
            so = so_full[:, :nt, :]
            for t in range(nt):
                nc.tensor.transpose(
                    so[:, t, :], s_sb[:, (t0 + t) * P : (t0 + t + 1) * P],
                    identity_bf[:],
                )
            return so

        def sq_part(c, so, t0=0, t1=4):
            # square (ACT); sum over DO (DVE)
            nt = t1 - t0
            sq = work.tile([P, nt, P], BF16, tag=f"sq{nt}")
            nc.scalar.square(sq[:], so[:])
            red = work.tile([P, nt * MO], BF16, tag=f"red{nt}")
            with nc.allow_low_precision(reason="squash norm tolerates bf16"):
                nc.vector.tensor_reduce(
                    red[:],
                    sq[:].rearrange("p t (g do) -> p (t g) do", do=DO),
                    axis=mybir.AxisListType.X,
                    op=mybir.AluOpType.add,
                )
            return red

        def post_a(c, s_sb, t0=0, t1=4):
            so = t_part(c, s_sb, t0, t1)
            red = sq_part(c, so, t0, t1)
            return so, red

        def post_b(c, so, red, t0=0, t1=4, tail=False, mul_engine=None, ring=None):
            # factor = red / ((1+red) * sqrt(red+eps)); scale + store.
            # The ACT sqrt sits two pipeline stages behind its reduce, so
            # neither ACT nor DVE ever stalls on the cross-engine hop.
            nt = t1 - t0
            r = work.tile([P, nt * MO], BF16, tag=f"r{nt}")
            nc.scalar.activation(
                r[:], red[:], mybir.ActivationFunctionType.Sqrt, bias=eps_sb[:]
            )
            d = work.tile([P, nt * MO], BF16, tag=f"d{nt}")
            nc.vector.scalar_tensor_tensor(
                d[:], red[:], 1.0, r[:], mybir.AluOpType.add, mybir.AluOpType.mult
            )
            rcp = work.tile([P, nt * MO], BF16, tag=f"rcp{nt}")
            with nc.allow_low_precision(reason="squash factor tolerates bf16"):
                nc.vector.reciprocal(rcp[:], d[:])
            fac = work.tile([P, nt * MO], BF16, tag=f"fac{nt}")
            nc.vector.tensor_mul(fac[:], red[:], rcp[:])
            # (Pool cannot read PSUM on TRN2, so the scale stays on DVE)
            (mul_engine or nc.vector).tensor_mul(
                out_sb[:, c, t0:t1].rearrange("p t (g do) -> p (t g) do", do=DO),
                so.rearrange("p t (g do) -> p (t g) do", do=DO),
                fac[:, :, None].to_broadcast((P, nt * MO, DO)),
            )
            # mid-stream stores ride the software DGE (Pool engine is idle
            # there and a SEQ-blocking wait is harmless); tail stores use
            # HWDGE rings (ACT/SP), which have no conflicting work left.
            if tail:
                (ring or nc.scalar).dma_start(out_d[:, c, t0:t1], out_sb[:, c, t0:t1])
            else:
                nc.gpsimd.dma_start(out_d[:, c, t0:t1], out_sb[:, c, t0:t1])

        def one_image():
            # Emission per chunk: mm(c), post_a(c-1), bias(c), post_b(c-1);
            # the last chunk posts in halves.
            load_group(0)
            load_group(1)
            wps = wpsum.tile([8, 64], F32)
            for _ in range(12):
                nc.tensor.matmul(wps[:], wrm[:, :8], wrm[:], start=True, stop=True)
            sbufs = {}  # c -> s_sb
            a_state = {}  # c -> (so, red)
            for c in range(NCHUNK - 1):
                if c == 1:
                    load_group(2)
                ps = conv_chunk(c)
                # post_a(c-1) BEFORE bias(c) on the ACT queue: T(c-1) runs
                # at mm(c)'s end, so square(c-1) is ready right away and
                # the DVE reduce isn't pushed a chunk late.  post_b(c-1)
                # directly after (depth-1 pipeline): the DVE absorbs the
                # sqrt round-trip mid-stream, so no post backlog remains
                # when the matmul stream drains.
                if c >= 1:
                    a_state[c - 1] = post_a(c - 1, sbufs.pop(c - 1))
                sbufs[c] = bias_chunk(ps)
                if c >= 1:
                    post_b(c - 1, *a_state.pop(c - 1))
            # drain: chunk 7 biases and posts in half-chunks so the tail's
            # serial chain is half-width all the way down.  The bias halves
            # go FIRST on the ACT queue (they gate the tail transposes);
            # squares/sqrts slot in behind them.
            ps7 = conv_chunk(7)
            so6 = t_part(6, sbufs.pop(6))
            s7 = bias_chunk(ps7, halves=True)
            red6 = sq_part(6, so6)
            so7a = t_part(7, s7, 0, 2)
            post_b(6, so6, red6, tail=True, ring=nc.scalar)
            red7a = sq_part(7, so7a, 0, 2)
            so7b = t_part(7, s7, 2, 4)
            post_b(7, so7a, red7a, 0, 2, tail=True, ring=nc.scalar)
            red7b = sq_part(7, so7b, 2, 4)
            post_b(7, so7b, red7b, 2, 4, tail=True, ring=nc.scalar)

        if reps == 1:
            one_image()
        else:
            with tc.For_i(0, reps, 1):
                one_image()


_NC_CACHE = {}


def _get_nc(reps=1):
    key = ("nc", reps)
    if key not in _NC_CACHE:
        nc = bacc.Bacc("TRN2", target_bir_lowering=False, debug=False, num_devices=8)
        x_in = nc.dram_tensor("x", [CI, H * 65], BF16, kind="ExternalInput").ap()
        w_in = nc.dram_tensor("w", [CI, 9, CO], BF16, kind="ExternalInput").ap()
        b_in = nc.dram_tensor("bias", [CO, 1], F32, kind="ExternalInput").ap()
        out_d = nc.dram_tensor("out", [P, NCHUNK, 4, CO], BF16, kind="ExternalOutput").ap()
        with tile.TileContext(nc) as tc:
            _body(tc, x_in, w_in, b_in, out_d, reps=reps)
        nc.compile()
        _NC_CACHE[key] = nc
    return _NC_CACHE[key]


def run(x, conv_w, conv_b, trace=False, reps=1):
    nc = _get_nc(reps=reps)
    # shard/prep: channel-major x per image with one zero column appended
    # per row (the device's 65-wide-row virtual-padding layout), bf16
    x4 = (
        np.asarray(x, dtype=np.float32)
        .transpose(0, 1, 4, 2, 3)
        .reshape(B, CI, H, W)
        .astype(ml_dtypes.bfloat16)
    )
    xt = np.zeros((B, CI, H, 65), dtype=ml_dtypes.bfloat16)
    xt[..., :W] = x4
    xt = np.ascontiguousarray(xt.reshape(B, CI, H * 65))
    w9 = np.ascontiguousarray(
        np.asarray(conv_w, dtype=np.float32)
        .reshape(CO, CI, 9)
        .transpose(1, 2, 0)
        .astype(ml_dtypes.bfloat16)
    )
    bias = np.ascontiguousarray(np.asarray(conv_b, dtype=np.float32).reshape(CO, 1))
    in_maps = [{"x": xt[b], "w": w9, "bias": bias} for b in range(B)]
    res = run_bass_kernel_spmd(nc, in_maps, list(range(B)), trace=trace)
    # gather/unshard: out_dev[p, c, t, mo, do] -> out[b, mo, h, w, do]
    # with h = 8c + 2t + p//64, w = p%64
    dev = np.stack(
        [res.results[i]["out"].astype(np.float32) for i in range(B)], axis=0
    )
    dev = dev.reshape(B, 2, W, NCHUNK, 4, MO, DO)  # [b, hl, w, c, t, mo, do]
    out = np.ascontiguousarray(
        dev.transpose(0, 5, 3, 4, 1, 2, 6).reshape(B, MO, H, W, DO)
    )
    return out, res


def kernel(x, conv_w, conv_b, b_logits=None, **_ignored):
    # b_logits provably has no effect on the reference output (see module
    # docstring), so it is accepted and ignored.
    out, _ = run(x, conv_w, conv_b, trace=False)
    return out
